# revision 11
# baseline (speedup 1.0000x reference)
"""Self-attention block (B=16, S=1024, C=512, H=8, D=64) on 8 NeuronCores.

Data-parallel over batch: core i handles batches [2i, 2i+1]. No collectives.

Per-core device pipeline (all on-chip after the initial DMAs):
  qkv proj -> q,k feature-major [d, s], v token-major [s, d] with a ones
  column appended per head (so P@V_ext also yields softmax row-sums);
  scores computed transposed S'[j, i] = k . q so exp(S') feeds the P@V
  matmul directly as lhsT with no transposes; softmax skips max-subtraction
  (logits bounded ~+-4, exact in fp32); deferred normalization divides
  O^T rows by the row-sum via a PE ones-broadcast + DVE multiply; output
  projection consumes the normalized heads straight out of SBUF.

The value-path bias is folded through attention into the output bias
(b_eff = b_out + b_v @ w_out.T), exact because softmax rows sum to 1.

Matmul operands are typed float32r (single-pass PE mode, 4x the fp32 rate);
set ATTN_MM_DT=f32 to fall back to full fp32 matmuls.
"""

import os
import numpy as np

import concourse.bacc as bacc
import concourse.tile as tile
import concourse.mybir as mybir
from concourse.bass_utils import run_bass_kernel_spmd

B, S, C, H, D = 16, 1024, 512, 8, 64
NCORES = 8
BPC = B // NCORES  # batches per core
F32 = mybir.dt.float32
# float32r: single-pass fp32 matmul (4x faster than float32 mode on the PE).
MDT = mybir.dt.float32r if os.environ.get("ATTN_MM_DT", "f32r") == "f32r" else F32

SCJ = 8  # S/128 chunks (token/key chunks)
CCH = 4  # C/128 chunks (model-dim chunks)
FCH = 8  # (2C)/128 chunks of q|k features
VW = H * (D + 1)  # 520: v row width incl. ones column per head


def _register_ntff_hook():
    # run_bass_kernel_spmd(trace=True) under axon needs antenv.axon_hooks,
    # which is absent in this image; register the equivalent hook directly.
    import sys, types

    if "antenv.axon_hooks" in sys.modules:
        return
    try:
        import trn_agent_boot.trn_boot as tb

        hook = [None]
        mod = types.ModuleType("antenv.axon_hooks")
        mod.set_axon_ntff_profile_hook = lambda h: hook.__setitem__(0, h)
        mod.get_axon_ntff_profile_hook = lambda: hook[0]
        sys.modules["antenv.axon_hooks"] = mod
        mod.set_axon_ntff_profile_hook(
            tb._ntff_profile_via_ctypes("/opt/axon/libaxon_pjrt.so")
        )
    except Exception:
        pass


def build():
    nc = bacc.Bacc("TRN2", target_bir_lowering=False, debug=False)

    xT = nc.declare_dram_parameter("xT", [BPC, C, S], MDT, isOutput=False)
    wqkvT = nc.declare_dram_parameter("wqkvT", [C, 3 * C], MDT, isOutput=False)
    wouT = nc.declare_dram_parameter("wouT", [C, C], MDT, isOutput=False)
    bqk = nc.declare_dram_parameter("bqk", [128, FCH], F32, isOutput=False)
    beff = nc.declare_dram_parameter("beff", [C], F32, isOutput=False)
    y = nc.declare_dram_parameter("y", [BPC, S, C], F32, isOutput=True)

    from contextlib import ExitStack

    with tile.TileContext(nc) as tc, ExitStack() as ctx:
        ctx.enter_context(
            nc.allow_low_precision(reason="float32r matmul operand staging")
        )
        consts = ctx.enter_context(tc.tile_pool(name="consts", bufs=1))
        xpool = ctx.enter_context(tc.tile_pool(name="x", bufs=2))
        qkpool = ctx.enter_context(tc.tile_pool(name="qkt", bufs=1))
        vpool = ctx.enter_context(tc.tile_pool(name="v", bufs=1))
        ppool = ctx.enter_context(tc.tile_pool(name="p", bufs=3))
        opool = ctx.enter_context(tc.tile_pool(name="o", bufs=1))
        rpool = ctx.enter_context(tc.tile_pool(name="r", bufs=2))
        ypool = ctx.enter_context(tc.tile_pool(name="y", bufs=2))
        bcpool = ctx.enter_context(tc.tile_pool(name="bc", bufs=2))
        drpool = ctx.enter_context(tc.tile_pool(name="dr", bufs=2, space="DRAM"))
        ps_a = ctx.enter_context(tc.tile_pool(name="ps_a", bufs=2, space="PSUM"))
        ps_o = ctx.enter_context(tc.tile_pool(name="ps_o", bufs=2, space="PSUM"))

        # --- constants ---
        wq_sb = consts.tile([128, CCH * 3 * C], MDT)  # [c%128, cc*1536 + f]
        nc.sync.dma_start(
            out=wq_sb.rearrange("p (cc f) -> p cc f", cc=CCH),
            in_=wqkvT[:, :].rearrange("(cc p) f -> p cc f", p=128),
        )
        wo_sb = consts.tile([128, CCH * C], MDT)  # [c%128, cc*512 + f]
        nc.sync.dma_start(
            out=wo_sb.rearrange("p (cc f) -> p cc f", cc=CCH),
            in_=wouT[:, :].rearrange("(cc p) f -> p cc f", p=128),
        )
        bqk_sb = consts.tile([128, FCH], F32)
        nc.sync.dma_start(out=bqk_sb, in_=bqk[:, :])
        beff_sb = consts.tile([128, C], F32)  # b_eff broadcast to all partitions
        nc.gpsimd.dma_start(out=beff_sb, in_=beff[:].partition_broadcast(128))


        for b in range(BPC):
            # --- load x^T for this batch: [c, s] as [c%128, cc*1024 + s] ---
            x_sb = xpool.tile([128, CCH * S], MDT)
            nc.sync.dma_start(
                out=x_sb.rearrange("p (cc s) -> p cc s", cc=CCH),
                in_=xT[b].rearrange("(cc p) s -> p cc s", p=128),
            )

            # --- q/k projection: qkT[f, s] = W_qk^T.T @ x^T + b, feature-major
            qkT_sb = qkpool.tile([128, FCH * S], MDT)  # [f%128, fc*1024 + s]
            for fc in range(FCH):
                ps = ps_a.tile([128, 1024], F32, tag="ps_a")
                for ih in range(2):
                    for cc in range(CCH):
                        nc.tensor.matmul(
                            ps[:, ih * 512 : (ih + 1) * 512],
                            lhsT=wq_sb[:, cc * 1536 + fc * 128 : cc * 1536 + (fc + 1) * 128],
                            rhs=x_sb[:, cc * S + ih * 512 : cc * S + ih * 512 + 512],
                            start=(cc == 0),
                            stop=(cc == CCH - 1),
                        )
                # evacuate + bias (per-partition scalar add)
                nc.vector.tensor_scalar_add(
                    out=qkT_sb[:, fc * S : (fc + 1) * S],
                    in0=ps[:, :],
                    scalar1=bqk_sb[:, fc : fc + 1],
                )

            # --- v projection: v[s, d] token-major, ones col per head ---
            v_sb = vpool.tile([128, SCJ * VW], MDT)  # [s%128, jc*520 + h*65 + d]
            v_view = v_sb.rearrange("p (jc h dd) -> p jc h dd", jc=SCJ, h=H)
            nc.gpsimd.memset(v_sb.bitcast(F32), 1.0)  # ones cols survive the data copies below
            for jc in range(SCJ):
                ps = ps_a.tile([128, 1024], F32, tag="ps_a")
                for cc in range(CCH):
                    nc.tensor.matmul(
                        ps[:, 0:512],
                        lhsT=x_sb[:, cc * S + jc * 128 : cc * S + (jc + 1) * 128],
                        rhs=wq_sb[:, cc * 1536 + 1024 : cc * 1536 + 1536],
                        start=(cc == 0),
                        stop=(cc == CCH - 1),
                    )
                nc.vector.tensor_copy(
                    out=v_view[:, jc, :, 0:D],
                    in_=ps[:, 0:512].rearrange("p (h d) -> p h d", h=H),
                )

            # --- attention per head ---
            o_sb = opool.tile([128, CCH * S], MDT)  # normalized heads, [c%128, cc*1024 + i]
            for h in range(H):
                fq = h // 2  # q features chunk
                fk = 4 + h // 2  # k features chunk
                pb = (h % 2) * 64  # partition base within chunk
                po = ps_o.tile([65, 1024], F32, tag="ps_o")
                for jc in range(SCJ):
                    # scores S'[j, i] = k . q  (transposed scores)
                    ps = ps_a.tile([128, 1024], F32, tag="ps_a")
                    for ih in range(2):
                        nc.tensor.matmul(
                            ps[:, ih * 512 : (ih + 1) * 512],
                            lhsT=qkT_sb[pb : pb + 64, fk * S + jc * 128 : fk * S + (jc + 1) * 128],
                            rhs=qkT_sb[pb : pb + 64, fq * S + ih * 512 : fq * S + ih * 512 + 512],
                            start=True,
                            stop=True,
                        )
                    # P' = exp(scale * S')
                    pt = ppool.tile([128, 1024], MDT, tag="p")
                    nc.scalar.activation(
                        out=pt, in_=ps[:, :],
                        func=mybir.ActivationFunctionType.Exp,
                        scale=float(D) ** -0.5,
                    )
                    # O^T[d, i] += V_ext^T @ P'  (row 64 = row-sums)
                    for ih in range(2):
                        nc.tensor.matmul(
                            po[:, ih * 512 : (ih + 1) * 512],
                            lhsT=v_sb[:, jc * VW + h * (D + 1) : jc * VW + (h + 1) * (D + 1)],
                            rhs=pt[:, ih * 512 : (ih + 1) * 512],
                            start=(jc == 0),
                            stop=(jc == SCJ - 1),
                        )
                # normalize: o = O^T[0:64] * (1 / rowsum) broadcast over partitions
                rec = rpool.tile([1, 1024], MDT, tag="r")
                nc.vector.reciprocal(out=rec, in_=po[64:65, :])
                rec_dr = drpool.tile([1, 1024], MDT, tag="dr")
                nc.gpsimd.dma_start(out=rec_dr, in_=rec)
                bc = bcpool.tile([64, 1024], MDT, tag="bc")
                nc.gpsimd.dma_start(out=bc, in_=rec_dr.partition_broadcast(64))
                nc.vector.tensor_mul(
                    out=o_sb[pb : pb + 64, (h // 2) * S : (h // 2 + 1) * S],
                    in0=po[0:64, :],
                    in1=bc[:, :],
                )

            # --- output projection + bias, then store ---
            y_sb = ypool.tile([128, SCJ * C], F32)  # [s%128, sc*512 + f]
            for sc in range(SCJ):
                ps = ps_a.tile([128, 1024], F32, tag="ps_a")
                for cc in range(CCH):
                    nc.tensor.matmul(
                        ps[:, 0:512],
                        lhsT=o_sb[:, cc * S + sc * 128 : cc * S + (sc + 1) * 128],
                        rhs=wo_sb[:, cc * C : (cc + 1) * C],
                        start=(cc == 0),
                        stop=(cc == CCH - 1),
                    )
                nc.vector.tensor_add(
                    out=y_sb[:, sc * C : (sc + 1) * C],
                    in0=ps[:, 0:512],
                    in1=beff_sb,
                )
            nc.sync.dma_start(
                out=y[b].rearrange("(sc p) f -> p sc f", p=128),
                in_=y_sb.rearrange("p (sc f) -> p sc f", sc=SCJ),
            )

    nc.compile()
    return nc


_NC_CACHE = None
LAST_RESULT = None


def kernel(vis_feat, text_feat, w_qkv, b_qkv, w_out, b_out):
    global _NC_CACHE, LAST_RESULT
    _register_ntff_hook()
    if _NC_CACHE is None:
        _NC_CACHE = build()
    nc = _NC_CACHE

    vis_feat = np.ascontiguousarray(vis_feat, dtype=np.float32)
    w_qkv = np.asarray(w_qkv, dtype=np.float32)
    b_qkv = np.asarray(b_qkv, dtype=np.float32)
    w_out = np.asarray(w_out, dtype=np.float32)
    b_out = np.asarray(b_out, dtype=np.float32)

    wqkvT = np.ascontiguousarray(w_qkv.T)  # [C, 3C]
    wouT = np.ascontiguousarray(w_out.T)  # [C, C]
    bqk = np.ascontiguousarray(b_qkv[: 2 * C].reshape(FCH, 128).T)  # [128, 8]
    beff = np.ascontiguousarray(b_out + b_qkv[2 * C :] @ w_out.T)  # [C]

    in_maps = []
    for i in range(NCORES):
        xT = np.ascontiguousarray(
            vis_feat[i * BPC : (i + 1) * BPC].transpose(0, 2, 1)
        )  # [BPC, C, S]
        in_maps.append(
            {"xT": xT, "wqkvT": wqkvT, "wouT": wouT, "bqk": bqk, "beff": beff}
        )

    res = run_bass_kernel_spmd(nc, in_maps, core_ids=list(range(NCORES)))
    LAST_RESULT = res
    return np.concatenate([res.results[i]["y"] for i in range(NCORES)], axis=0)


# revision 20
# speedup vs baseline: 1.2214x; 1.2214x over previous
"""Self-attention block (B=16, S=1024, C=512, H=8, D=64) on 8 NeuronCores.

Data-parallel over batch: core i handles batches [2i, 2i+1]. No collectives.

Per-core device pipeline (all on-chip after the initial DMAs):
  qkv proj -> q,k feature-major [d, s], v token-major [s, d] with a ones
  column appended per head (so P@V_ext also yields softmax row-sums);
  scores computed transposed S'[j, i] = k . q so exp(S') feeds the P@V
  matmul directly as lhsT with no transposes; softmax skips max-subtraction
  (logits bounded ~+-4, exact in fp32); deferred normalization divides
  O^T rows by the row-sum via a PE ones-broadcast + DVE multiply; output
  projection consumes the normalized heads straight out of SBUF.

The value-path bias is folded through attention into the output bias
(b_eff = b_out + b_v @ w_out.T), exact because softmax rows sum to 1.

Matmul operands are typed float32r (single-pass PE mode, 4x the fp32 rate);
set ATTN_MM_DT=f32 to fall back to full fp32 matmuls.
"""

import os
import numpy as np

import concourse.bacc as bacc
import concourse.tile as tile
import concourse.mybir as mybir
from concourse.bass_utils import run_bass_kernel_spmd

B, S, C, H, D = 16, 1024, 512, 8, 64
NCORES = 8
BPC = B // NCORES  # batches per core
F32 = mybir.dt.float32
# float32r: single-pass fp32 matmul (4x faster than float32 mode on the PE).
MDT = mybir.dt.float32r if os.environ.get("ATTN_MM_DT", "f32r") == "f32r" else F32

SCJ = 8  # S/128 chunks (token/key chunks)
CCH = 4  # C/128 chunks (model-dim chunks)
FCH = 8  # (2C)/128 chunks of q|k features
VW = H * (D + 1)  # 520: v row width incl. ones column per head


def _register_ntff_hook():
    # run_bass_kernel_spmd(trace=True) under axon needs antenv.axon_hooks,
    # which is absent in this image; register the equivalent hook directly.
    import sys, types

    if "antenv.axon_hooks" in sys.modules:
        return
    try:
        import trn_agent_boot.trn_boot as tb

        hook = [None]
        mod = types.ModuleType("antenv.axon_hooks")
        mod.set_axon_ntff_profile_hook = lambda h: hook.__setitem__(0, h)
        mod.get_axon_ntff_profile_hook = lambda: hook[0]
        sys.modules["antenv.axon_hooks"] = mod
        mod.set_axon_ntff_profile_hook(
            tb._ntff_profile_via_ctypes("/opt/axon/libaxon_pjrt.so")
        )
    except Exception:
        pass


def build():
    nc = bacc.Bacc("TRN2", target_bir_lowering=False, debug=False)

    xT = nc.declare_dram_parameter("xT", [BPC, C, S], MDT, isOutput=False)
    wqkvT = nc.declare_dram_parameter("wqkvT", [C, 3 * C], MDT, isOutput=False)
    wouT = nc.declare_dram_parameter("wouT", [C, C], MDT, isOutput=False)
    bqk = nc.declare_dram_parameter("bqk", [128, FCH], F32, isOutput=False)
    beff = nc.declare_dram_parameter("beff", [C], F32, isOutput=False)
    y = nc.declare_dram_parameter("y", [BPC, S, C], F32, isOutput=True)

    from contextlib import ExitStack

    with tile.TileContext(nc) as tc, ExitStack() as ctx:
        ctx.enter_context(
            nc.allow_low_precision(reason="float32r matmul operand staging")
        )
        consts = ctx.enter_context(tc.tile_pool(name="consts", bufs=1))
        xpool = ctx.enter_context(tc.tile_pool(name="x", bufs=2))
        qkpool = ctx.enter_context(tc.tile_pool(name="qkt", bufs=1))
        vpool = ctx.enter_context(tc.tile_pool(name="v", bufs=1))
        ppool = ctx.enter_context(tc.tile_pool(name="p", bufs=3))
        opool = ctx.enter_context(tc.tile_pool(name="o", bufs=1))
        rpool = ctx.enter_context(tc.tile_pool(name="r", bufs=2))
        spool = ctx.enter_context(tc.tile_pool(name="s", bufs=1))
        ypool = ctx.enter_context(tc.tile_pool(name="y", bufs=1))
        bcpool = ctx.enter_context(tc.tile_pool(name="bc", bufs=2))
        drpool = ctx.enter_context(tc.tile_pool(name="dr", bufs=2, space="DRAM"))
        ps_a = ctx.enter_context(tc.tile_pool(name="ps_a", bufs=3, space="PSUM"))
        ps_o = ctx.enter_context(tc.tile_pool(name="ps_o", bufs=1, space="PSUM"))

        # --- constants ---
        wq_sb = consts.tile([128, CCH * 3 * C], MDT)  # [c%128, cc*1536 + f]
        nc.sync.dma_start(
            out=wq_sb.rearrange("p (cc f) -> p cc f", cc=CCH),
            in_=wqkvT[:, :].rearrange("(cc p) f -> p cc f", p=128),
        )
        wo_sb = consts.tile([128, CCH * C], MDT)  # [c%128, cc*512 + f]
        nc.sync.dma_start(
            out=wo_sb.rearrange("p (cc f) -> p cc f", cc=CCH),
            in_=wouT[:, :].rearrange("(cc p) f -> p cc f", p=128),
        )
        bqk_sb = consts.tile([128, FCH], F32)
        nc.sync.dma_start(out=bqk_sb, in_=bqk[:, :])
        beff_sb = consts.tile([128, C], F32)  # b_eff broadcast to all partitions
        nc.gpsimd.dma_start(out=beff_sb, in_=beff[:].partition_broadcast(128))


        for b in range(BPC):
            # --- load x^T for this batch: [c, s] as [c%128, cc*1024 + s] ---
            x_sb = xpool.tile([128, CCH * S], MDT)
            nc.sync.dma_start(
                out=x_sb.rearrange("p (cc s) -> p cc s", cc=CCH),
                in_=xT[b].rearrange("(cc p) s -> p cc s", p=128),
            )

            # --- q/k projection: qkT[f, s] = W_qk^T.T @ x^T + b, feature-major
            qkT_sb = qkpool.tile([128, FCH * S], MDT)  # [f%128, fc*1024 + s]
            for fc in range(FCH):
                ps = ps_a.tile([128, 1024], F32, tag="ps_a")
                for ih in range(2):
                    for cc in range(CCH):
                        nc.tensor.matmul(
                            ps[:, ih * 512 : (ih + 1) * 512],
                            lhsT=wq_sb[:, cc * 1536 + fc * 128 : cc * 1536 + (fc + 1) * 128],
                            rhs=x_sb[:, cc * S + ih * 512 : cc * S + ih * 512 + 512],
                            start=(cc == 0),
                            stop=(cc == CCH - 1),
                        )
                # evacuate + bias (per-partition scalar add)
                nc.vector.tensor_scalar_add(
                    out=qkT_sb[:, fc * S : (fc + 1) * S],
                    in0=ps[:, :],
                    scalar1=bqk_sb[:, fc : fc + 1],
                )

            # --- v projection: v[s, d] token-major, ones col per head ---
            v_sb = vpool.tile([128, SCJ * VW], MDT)  # [s%128, jc*520 + h*65 + d]
            v_view = v_sb.rearrange("p (jc h dd) -> p jc h dd", jc=SCJ, h=H)
            nc.gpsimd.memset(v_sb.bitcast(F32), 1.0)  # ones cols survive the data copies below
            for jc in range(SCJ):
                ps = ps_a.tile([128, 1024], F32, tag="ps_a")
                for cc in range(CCH):
                    nc.tensor.matmul(
                        ps[:, 0:512],
                        lhsT=x_sb[:, cc * S + jc * 128 : cc * S + (jc + 1) * 128],
                        rhs=wq_sb[:, cc * 1536 + 1024 : cc * 1536 + 1536],
                        start=(cc == 0),
                        stop=(cc == CCH - 1),
                    )
                nc.vector.tensor_copy(
                    out=v_view[:, jc, :, 0:D],
                    in_=ps[:, 0:512].rearrange("p (h d) -> p h d", h=H),
                )

            # --- attention per head ---
            o_sb = opool.tile([128, CCH * S], MDT)  # heads, [c%128, cc*1024 + i]
            # head h's row-sums live on partition 32*(h%4), col block h//4
            sums_sb = spool.tile([97, 2 * S], F32, tag="sums")
            for h in range(H):
                fq = h // 2  # q features chunk
                fk = 4 + h // 2  # k features chunk
                pb = (h % 2) * 64  # partition base within chunk
                po = ps_o.tile([65, 1024], F32, tag="ps_o")
                for jc in range(SCJ):
                    # scores S'[j, i] = k . q  (transposed scores)
                    ps = ps_a.tile([128, 1024], F32, tag="ps_a")
                    for ih in range(2):
                        nc.tensor.matmul(
                            ps[:, ih * 512 : (ih + 1) * 512],
                            lhsT=qkT_sb[pb : pb + 64, fk * S + jc * 128 : fk * S + (jc + 1) * 128],
                            rhs=qkT_sb[pb : pb + 64, fq * S + ih * 512 : fq * S + ih * 512 + 512],
                            start=True,
                            stop=True,
                        )
                    # P' = exp(scale * S')
                    pt = ppool.tile([128, 1024], MDT, tag="p")
                    nc.scalar.activation(
                        out=pt, in_=ps[:, :],
                        func=mybir.ActivationFunctionType.Exp,
                        scale=float(D) ** -0.5,
                    )
                    # O^T[d, i] += V_ext^T @ P'  (row 64 = row-sums)
                    for ih in range(2):
                        nc.tensor.matmul(
                            po[:, ih * 512 : (ih + 1) * 512],
                            lhsT=v_sb[:, jc * VW + h * (D + 1) : jc * VW + (h + 1) * (D + 1)],
                            rhs=pt[:, ih * 512 : (ih + 1) * 512],
                            start=(jc == 0),
                            stop=(jc == SCJ - 1),
                        )
                # evacuate unnormalized O^T and its row-sums (frees PSUM fast;
                # the expensive reciprocal is batched once per batch below)
                nc.vector.tensor_copy(
                    out=o_sb[pb : pb + 64, (h // 2) * S : (h // 2 + 1) * S],
                    in_=po[0:64, :],
                )
                pa = 32 * (h % 4)
                nc.scalar.copy(
                    out=sums_sb[pa : pa + 1, (h // 4) * S : (h // 4 + 1) * S],
                    in_=po[64:65, :],
                )

            # --- batched softmax normalization for all 8 heads ---
            # Bounce sums through DRAM to respread into [128, 64] so the
            # (expensive, ~8 cyc/elem) reciprocal runs across all 128 lanes.
            # DRAM order: head h at offset ((h%4)*2 + h//4) * S.
            sums_dr = drpool.tile([H * S], F32, tag="sdr")
            for a in range(4):
                nc.gpsimd.dma_start(
                    out=sums_dr[a * 2 * S : (a + 1) * 2 * S].unsqueeze(0),
                    in_=sums_sb[32 * a : 32 * a + 1, :],
                )
            sums_sq = rpool.tile([128, H * S // 128], F32, tag="ssq")
            nc.gpsimd.dma_start(
                out=sums_sq, in_=sums_dr.rearrange("(p c) -> p c", p=128)
            )
            recs_sq = rpool.tile([128, H * S // 128], F32, tag="rsq")
            nc.vector.reciprocal(out=recs_sq, in_=sums_sq)
            recs_dr = drpool.tile([H * S], F32, tag="rdr")
            nc.gpsimd.dma_start(
                out=recs_dr.rearrange("(p c) -> p c", p=128), in_=recs_sq
            )
            for cc in range(CCH):
                bc = bcpool.tile([128, S], F32, tag="bc")
                for hh in range(2):
                    h = 2 * cc + hh
                    off = ((h % 4) * 2 + h // 4) * S
                    nc.gpsimd.dma_start(
                        out=bc[hh * 64 : (hh + 1) * 64, :],
                        in_=recs_dr[off : off + S].partition_broadcast(64),
                    )
                nc.vector.tensor_mul(
                    out=o_sb[:, cc * S : (cc + 1) * S],
                    in0=o_sb[:, cc * S : (cc + 1) * S],
                    in1=bc,
                )

            # --- output projection + bias, then store ---
            y_sb = ypool.tile([128, SCJ * C], F32)  # [s%128, sc*512 + f]
            for sc in range(SCJ):
                ps = ps_a.tile([128, 1024], F32, tag="ps_a")
                for cc in range(CCH):
                    nc.tensor.matmul(
                        ps[:, 0:512],
                        lhsT=o_sb[:, cc * S + sc * 128 : cc * S + (sc + 1) * 128],
                        rhs=wo_sb[:, cc * C : (cc + 1) * C],
                        start=(cc == 0),
                        stop=(cc == CCH - 1),
                    )
                nc.vector.tensor_add(
                    out=y_sb[:, sc * C : (sc + 1) * C],
                    in0=ps[:, 0:512],
                    in1=beff_sb,
                )
            nc.sync.dma_start(
                out=y[b].rearrange("(sc p) f -> p sc f", p=128),
                in_=y_sb.rearrange("p (sc f) -> p sc f", sc=SCJ),
            )

    nc.compile()
    return nc


_NC_CACHE = None
LAST_RESULT = None


def kernel(vis_feat, text_feat, w_qkv, b_qkv, w_out, b_out):
    global _NC_CACHE, LAST_RESULT
    _register_ntff_hook()
    if _NC_CACHE is None:
        _NC_CACHE = build()
    nc = _NC_CACHE

    vis_feat = np.ascontiguousarray(vis_feat, dtype=np.float32)
    w_qkv = np.asarray(w_qkv, dtype=np.float32)
    b_qkv = np.asarray(b_qkv, dtype=np.float32)
    w_out = np.asarray(w_out, dtype=np.float32)
    b_out = np.asarray(b_out, dtype=np.float32)

    wqkvT = np.ascontiguousarray(w_qkv.T)  # [C, 3C]
    wouT = np.ascontiguousarray(w_out.T)  # [C, C]
    bqk = np.ascontiguousarray(b_qkv[: 2 * C].reshape(FCH, 128).T)  # [128, 8]
    beff = np.ascontiguousarray(b_out + b_qkv[2 * C :] @ w_out.T)  # [C]

    in_maps = []
    for i in range(NCORES):
        xT = np.ascontiguousarray(
            vis_feat[i * BPC : (i + 1) * BPC].transpose(0, 2, 1)
        )  # [BPC, C, S]
        in_maps.append(
            {"xT": xT, "wqkvT": wqkvT, "wouT": wouT, "bqk": bqk, "beff": beff}
        )

    res = run_bass_kernel_spmd(nc, in_maps, core_ids=list(range(NCORES)))
    LAST_RESULT = res
    return np.concatenate([res.results[i]["y"] for i in range(NCORES)], axis=0)
